# revision 20
# baseline (speedup 1.0000x reference)
"""Trainium2 Bass kernel (bf16 matmul operands, fp32 PSUM accumulation) for multi-head attention (nn_Attention_24764781428921).

Reference (fp32):
    q = heads(x @ Wq + bq); k = heads(x @ Wk + bk); v = heads(x @ Wv + bv)
    probs = softmax(q k^T / sqrt(1024)); ctx = probs @ v
    out = unheads(ctx) @ Wo + bo
with x [2, 2048, 1024], 16 heads, head_dim 64.

Sharding: DP=2 over batch x TP=4 over heads (4 heads / 256 channels per core).
Each core returns a partial [2048, 1024] = ctx_local @ Wo[local_rows]; the host
sums the 4 TP partials per batch and adds bo + bv @ Wo (the bv contribution is
linear, so it is folded on the host: out += bo + bv @ Wo).

On-chip layout (per core):
  xT [1024, 2048]  (host pre-transposed)  -> SBUF [128, 8ct, 2048]
  qT/kT = W^T xT + b : [256, 2048] as [128, 2dt, 2048]   (channel on partitions)
  v     = xT^T Wv    : [2048, 256] -> vaug [128, 16jt, 4h, 65] (col 64 = ones)
  s^T(h, jt, ib) [128j, 512i] = kT_h[:, jt]^T-free q: matmul(lhsT=kT slice, rhs=qT slice)
     head pairs row-packed (K=64 at partition offsets 0/64 run concurrently)
  exp on ScalarE with scale=1/32 folded in (no max subtraction: |s|<~2)
  ctx^T+sums [65, 512] accumulated over 16 jt (ones row gives softmax denom)
  normalize: recip(sums) -> PE partition-broadcast [64,512]; mul
  out[i, o] partial: matmul(lhsT=ctxn^T [c,128i], rhs=Wo [c, 512o]) accum 2 ct

Scheduling (v2): the kernel is ScalarE(exp)-bound in steady state (~1.15us per
jt tile of 128x1024 scores).  The ramp is minimized by starting attention as
soon as k/q projections for the first i-block finish (per-ib projection chains
instead of ct-interleaved all-ib chains, and xT DMA split so the first 512
columns of every c-tile arrive first).  All remaining projections (q ib1-3,
k/q dt1) and the v projection are emitted via per-jt hooks inside the attention
blocks so the list scheduler drops them into PE slack under the exp stream.
"""

import numpy as np

HID = 1024
N = 2048
DL = 256          # local channels per core (4 heads x 64)
NHEAD = 4         # local heads
HD = 64
CT = HID // 128   # 8 c-tiles
DT = DL // 128    # 2 d-tiles
JT = N // 128     # 16 j-tiles
IB = N // 512     # 4 i-blocks
SCALE = 1.0 / 32.0  # 1/sqrt(1024)

_prog_cache = {}


def build_program(reps=1, loop_reps=None):
    import concourse.bass as bass
    import concourse.mybir as mybir
    import concourse.tile as tile
    from concourse import bacc

    F32 = mybir.dt.float32
    F32R = mybir.dt.bfloat16  # matmul operand dtype: bf16 gets FWL weight loads + 1 cyc/row
    AF = mybir.ActivationFunctionType

    def mm(out, lhsT, rhs, **kw):
        nc.tensor.matmul(out, lhsT=lhsT, rhs=rhs, **kw)

    nc = bacc.Bacc()
    xT = nc.dram_tensor("xT", [HID, N], F32R, kind="ExternalInput")
    wq = nc.dram_tensor("wq", [HID, DL], F32R, kind="ExternalInput")
    wk = nc.dram_tensor("wk", [HID, DL], F32R, kind="ExternalInput")
    wv = nc.dram_tensor("wv", [HID, DL], F32R, kind="ExternalInput")
    wo = nc.dram_tensor("wo", [DL, HID], F32R, kind="ExternalInput")
    bq = nc.dram_tensor("bq", [128, DT], F32, kind="ExternalInput")
    bk = nc.dram_tensor("bk", [128, DT], F32, kind="ExternalInput")
    y = nc.dram_tensor("y", [N, HID], F32, kind="ExternalOutput")

    with tile.TileContext(nc) as tc:
        with (
            tc.tile_pool(name="consts", bufs=1) as consts,
            tc.tile_pool(name="qkv_sb", bufs=1) as qkv_sb,
            tc.tile_pool(name="exp_sb", bufs=8) as exp_pool,
            tc.tile_pool(name="ctxn_sb", bufs=4) as ctxn_pool,
            tc.tile_pool(name="bcast_sb", bufs=4) as bcast_pool,
            tc.tile_pool(name="rec_sb", bufs=4) as rec_pool,
            tc.tile_pool(name="out_sb", bufs=4) as out_pool,
        ):
            from contextlib import nullcontext
            loop_cm = tc.For_i(0, loop_reps, 1) if loop_reps is not None else nullcontext()
            with loop_cm:
              for rep in range(reps):
                  # ---- constants: DMA order is the ramp.  wk/wq then the xt
                  # c-tiles on the HWDGE queue (the ramp chains pipeline with
                  # the arriving tiles); wv/biases/wo go on the SWDGE queue so
                  # they land early without delaying the xt stream. ----
                  wk_sb = consts.tile([128, CT, DL], F32R)
                  wq_sb = consts.tile([128, CT, DL], F32R)
                  xt_sb = consts.tile([128, CT, N], F32R)
                  xt_r = xT[:, :].rearrange("(ct p) i -> p ct i", p=128)
                  wv_sb = consts.tile([128, CT, DL], F32R)
                  nc.gpsimd.dma_start(out=wv_sb, in_=wv[:, :].rearrange("(ct p) d -> p ct d", p=128))
                  bq_sb = consts.tile([128, DT], F32)
                  nc.gpsimd.dma_start(out=bq_sb, in_=bq[:, :])
                  bk_sb = consts.tile([128, DT], F32)
                  nc.gpsimd.dma_start(out=bk_sb, in_=bk[:, :])
                  wo_sb = consts.tile([128, DT, HID], F32R)
                  nc.sync.dma_start(out=wk_sb, in_=wk[:, :].rearrange("(ct p) d -> p ct d", p=128))
                  nc.sync.dma_start(out=wq_sb, in_=wq[:, :].rearrange("(ct p) d -> p ct d", p=128))
                  for ct in range(CT):
                      nc.sync.dma_start(out=xt_sb[:, ct, :], in_=xt_r[:, ct, :])

                  ones_sb = consts.tile([1, HD], F32R)
                  nc.vector.memset(ones_sb, 1.0)

                  qT = qkv_sb.tile([128, DT, N], F32R, tag="qT")
                  kT = qkv_sb.tile([128, DT, N], F32R, tag="kT")
                  vaug = qkv_sb.tile([128, JT, NHEAD, HD + 1], F32R, tag="vaug")
                  nc.vector.memset(vaug[:, :, :, HD:HD + 1], 1.0)

                  # Granules: small closures each emitting <= ~900ns of PE work,
                  # consumed one per jt period inside the attention blocks so
                  # the exp stream on ScalarE is never starved of scores.
                  def make_proj_halves(w_sb, b_sb, dest, dt, ib, ps_aux, pname):
                      """q/k projection for one (dt, ib) as two 4-ct half-chains."""
                      state = {}

                      def part_a():
                          ps = ps_aux.tile([128, 512], F32, tag="aux",
                                           name=f"r{rep}_{pname}_{dt}_{ib}")
                          state["ps"] = ps
                          for ct in range(CT // 2):
                              mm(ps, w_sb[:, ct, dt * 128:(dt + 1) * 128],
                                 xt_sb[:, ct, ib * 512:(ib + 1) * 512],
                                 start=(ct == 0), stop=False)

                      def part_b():
                          ps = state["ps"]
                          for ct in range(CT // 2, CT):
                              mm(ps, w_sb[:, ct, dt * 128:(dt + 1) * 128],
                                 xt_sb[:, ct, ib * 512:(ib + 1) * 512],
                                 start=False, stop=(ct == CT - 1))
                          nc.vector.tensor_scalar_add(
                              dest[:, dt, ib * 512:(ib + 1) * 512], ps,
                              b_sb[:, dt:dt + 1])

                      return part_a, part_b

                  def make_v(jt, ps_aux):
                      """v[j, d] = xT^T Wv for one j-tile (bias folded on host)."""
                      def g():
                          psv = ps_aux.tile([128, DL], F32, tag="aux",
                                            name=f"r{rep}_psv_{jt}")
                          for ct in range(CT):
                              mm(psv, xt_sb[:, ct, jt * 128:(jt + 1) * 128],
                                 wv_sb[:, ct, :],
                                 start=(ct == 0), stop=(ct == CT - 1))
                          nc.vector.tensor_copy(
                              out=vaug[:, jt, :, 0:HD],
                              in_=psv.rearrange("p (h d) -> p h d", h=NHEAD))
                      return g

                  def make_norm(ib, hp, ctxn, ctx_ps, ps_aux):
                      """Deferred per-u softmax normalization of a finished block."""
                      def norm(u):
                          poff = u * HD
                          # softmax denominators came out in the ones-row (row HD)
                          rec = rec_pool.tile([1, 512], F32R, tag="rec",
                                              name=f"r{rep}_rec_{ib}_{hp}_{u}")
                          with nc.allow_low_precision(reason="f32r is fp32-width; matmul input rounding"):
                              nc.vector.reciprocal(rec, ctx_ps[u][HD:HD + 1, :])
                          # partition-broadcast via K=1 outer product on the PE
                          bc_ps = ps_aux.tile([HD, 512], F32, tag="aux",
                                              name=f"r{rep}_bcps_{ib}_{hp}_{u}")
                          mm(bc_ps, ones_sb, rec, start=True, stop=True)
                          bc = bcast_pool.tile([HD, 512], F32, tag="bc",
                                               name=f"r{rep}_bc_{ib}_{hp}_{u}")
                          nc.vector.tensor_copy(out=bc, in_=bc_ps)
                          # ctxn = ctx * recip   (bv folded on host)
                          nc.vector.tensor_mul(
                              ctxn[poff:poff + HD, hp, :], ctx_ps[u][0:HD, :], bc)
                      return norm

                  def make_wo_granules(ib, ctxn, ps_aux):
                      """Wo for one i-block as 8 granules (one per (it, ot));
                      the output store fires after each it pair."""
                      obs = {}

                      def make(it, ot):
                          def g():
                              if ot == 0:
                                  obs[it] = out_pool.tile([128, 2, 512], F32, tag="ob",
                                                          name=f"r{rep}_ob_{ib}_{it}")
                              pso = ps_aux.tile([128, 512], F32, tag="aux",
                                                name=f"r{rep}_pso_{ib}_{it}_{ot}")
                              for ct in range(DT):
                                  mm(pso, ctxn[:, ct, it * 128:(it + 1) * 128],
                                     wo_sb[:, ct, ot * 512:(ot + 1) * 512],
                                     start=(ct == 0), stop=(ct == DT - 1))
                              nc.vector.tensor_copy(out=obs[it][:, ot, :], in_=pso)
                              if ot == 1:
                                  nc.gpsimd.dma_start(
                                      out=y[ib * 512 + it * 128:ib * 512 + (it + 1) * 128, :],
                                      in_=obs[it].rearrange("p a b -> p (a b)"))
                          return g

                      return [make(it, ot) for it in range(4) for ot in range(2)]

                  def emit_ctx(ib, hp, ctx_ps, ex, jt):
                      for u in range(2):
                          h = 2 * hp + u
                          mm(
                              ctx_ps[u],
                              vaug[:, jt, h, :],
                              ex[:, u, :],
                              start=(jt == 0),
                              stop=(jt == JT - 1),
                          )

                  def attn_block(ib, hp, ctxn, ps_s, ps_ctx, ps_aux,
                                 granules=(), prev_norm=None):
                      """Scores + exp + ctx accumulate for one head pair.

                      Emission is software-pipelined one jt ahead so the PE stream
                      never waits in-line on the exp of the current jt.  One filler
                      granule per period; the previous block's normalization is
                      emitted at jt==1/2.  Returns this block's deferred norm.
                      """
                      isl = slice(ib * 512, (ib + 1) * 512)
                      # ctx tiles are allocated lazily at the first emit_ctx so
                      # the slot-reuse WAR sees the previous block's (deferred)
                      # normalize reads, which are emitted at jt==1 below.
                      ctx_ps = []
                      gi = iter(granules)
                      exs = {}
                      for jt in range(JT):
                          s_ps = ps_s.tile([128, 2, 512], F32, tag="s", name=f"r{rep}_s_{ib}_{hp}_{jt}")
                          for u in range(2):
                              poff = u * HD
                              mm(
                                  s_ps[:, u, :],
                                  kT[poff:poff + HD, hp, jt * 128:(jt + 1) * 128],
                                  qT[poff:poff + HD, hp, isl],
                                  start=True,
                                  stop=True,
                              )
                          ex = exp_pool.tile([128, 2, 512], F32R, tag="ex", name=f"r{rep}_ex_{ib}_{hp}_{jt}")
                          nc.scalar.activation(ex, s_ps, AF.Exp, scale=SCALE)
                          exs[jt] = ex
                          if jt == 1:
                              if prev_norm is not None:
                                  prev_norm(0)
                                  prev_norm(1)
                              ctx_ps.extend(
                                  ps_ctx.tile([HD + 1, 512], F32, tag="ctx",
                                              name=f"r{rep}_ctx_{ib}_{hp}_{u}")
                                  for u in range(2)
                              )
                          g = next(gi, None)
                          if g is not None:
                              g()
                          if jt > 0:
                              emit_ctx(ib, hp, ctx_ps, exs.pop(jt - 1), jt - 1)
                      emit_ctx(ib, hp, ctx_ps, exs.pop(JT - 1), JT - 1)
                      for g in gi:
                          g()
                      return make_norm(ib, hp, ctxn, ctx_ps, ps_aux)

                  # ---- PSUM budget: ps_s 2x2 + ps_ctx 2x1 + ps_aux 2x1 = 8 banks.
                  # ps_aux is a shared 2-slot ring for every small PSUM user
                  # (projection half-chains, v chains, bcast outer products, Wo).
                  # During the DMA-paced ramp the two idle s-slots moonlight as
                  # v accumulators for j-tiles 0..7 (evacuated right before the
                  # first scores reuse them). ----
                  with (
                      tc.tile_pool(name="ps_s", bufs=2, space="PSUM") as ps_s,
                      tc.tile_pool(name="ps_ctx", bufs=2, space="PSUM") as ps_ctx,
                      tc.tile_pool(name="ps_aux", bufs=2, space="PSUM") as ps_aux,
                  ):
                      # Ramp: k(ib0), q(ib0) and v(0..5) chains all pipeline
                      # per-ct with the arriving xt c-tiles.  A PSUM
                      # accumulation chain owns its whole bank (start=True
                      # zeroes the bank), so each v chain gets a private bank:
                      # two banks per borrowed s-slot, one per ctx slot.
                      ps_k0 = ps_aux.tile([128, 512], F32, tag="aux", name=f"r{rep}_k_0_0")
                      ps_q0 = ps_aux.tile([128, 512], F32, tag="aux", name=f"r{rep}_q_0_0")
                      vv = [
                          ps_s.tile([128, 2, 512], F32, tag="s", name=f"r{rep}_vv{h}")
                          for h in range(2)
                      ]
                      vc = [
                          ps_ctx.tile([128, DL], F32, tag="ctx", name=f"r{rep}_vc{h}")
                          for h in range(2)
                      ]
                      ramp_v = [vv[0][:, 0, 0:DL], vv[0][:, 1, 0:DL],
                                vv[1][:, 0, 0:DL], vv[1][:, 1, 0:DL],
                                vc[0], vc[1]]
                      for ct in range(CT):
                          mm(ps_k0, wk_sb[:, ct, 0:128], xt_sb[:, ct, 0:512],
                             start=(ct == 0), stop=(ct == CT - 1))
                          mm(ps_q0, wq_sb[:, ct, 0:128], xt_sb[:, ct, 0:512],
                             start=(ct == 0), stop=(ct == CT - 1))
                          for jt in range(6):
                              mm(ramp_v[jt],
                                 xt_sb[:, ct, jt * 128:(jt + 1) * 128],
                                 wv_sb[:, ct, :],
                                 start=(ct == 0), stop=(ct == CT - 1))
                      nc.vector.tensor_scalar_add(kT[:, 0, 0:512], ps_k0, bk_sb[:, 0:1])
                      nc.vector.tensor_scalar_add(qT[:, 0, 0:512], ps_q0, bq_sb[:, 0:1])
                      for jt in range(6):
                          nc.vector.tensor_copy(
                              out=vaug[:, jt, :, 0:HD],
                              in_=ramp_v[jt].rearrange("p (h d) -> p h d", h=NHEAD))

                      ctxns = {
                          ib: ctxn_pool.tile([128, DT, 512], F32R, tag="ctxn",
                                             name=f"r{rep}_ctxn_{ib}")
                          for ib in range(IB)
                      }

                      def proj_g(pname, dt, ib):
                          w_sb, b_sb, dest = ((wk_sb, bk_sb, kT) if pname == "k"
                                              else (wq_sb, bq_sb, qT))
                          return make_proj_halves(w_sb, b_sb, dest, dt, ib, ps_aux, pname)

                      k1a, k1b = proj_g("k", 0, 1)
                      k2a, k2b = proj_g("k", 0, 2)
                      k3a, k3b = proj_g("k", 0, 3)
                      q1a, q1b = proj_g("q", 0, 1)
                      q2a, q2b = proj_g("q", 0, 2)
                      q3a, q3b = proj_g("q", 0, 3)
                      kd = [proj_g("k", 1, ib) for ib in range(IB)]
                      qd = [proj_g("q", 1, ib) for ib in range(IB)]
                      vs = {jt: make_v(jt, ps_aux) for jt in range(6, JT)}

                      # granule lists per hp0 block (deadline-ordered; the first
                      # two block0 periods take doubled granules -- no ctx work
                      # competes for the PE yet)
                      def pair(a, b):
                          def g():
                              a(); b()
                          return g

                      g0 = [pair(k1a, k1b), pair(vs[6], vs[7]), k2a, k2b,
                            vs[8], vs[9], k3a, k3b, vs[10], vs[11],
                            vs[12], vs[13], vs[14], vs[15], q1a, q1b]
                      g1 = [kd[0][0], kd[0][1], kd[1][0], kd[1][1],
                            kd[2][0], kd[2][1], kd[3][0], kd[3][1],
                            q2a, q2b, q3a, q3b]
                      g2 = [qd[0][0], qd[0][1], qd[1][0], qd[1][1],
                            qd[2][0], qd[2][1], qd[3][0], qd[3][1]]

                      norm = attn_block(0, 0, ctxns[0], ps_s, ps_ctx, ps_aux, granules=g0)
                      # wo lands well before the hp1 blocks; issuing it here keeps
                      # its transfer out of the ramp's xt stream.
                      nc.gpsimd.dma_start(out=wo_sb, in_=wo[:, :].rearrange("(ct p) o -> p ct o", p=128))
                      norm = attn_block(1, 0, ctxns[1], ps_s, ps_ctx, ps_aux,
                                        granules=g1, prev_norm=norm)
                      norm = attn_block(2, 0, ctxns[2], ps_s, ps_ctx, ps_aux,
                                        granules=g2, prev_norm=norm)
                      norm = attn_block(3, 0, ctxns[3], ps_s, ps_ctx, ps_aux,
                                        prev_norm=norm)

                      for ib in range(IB):
                          # Wo(ib-1) needs ctxn[ib-1] dt1, normalized at jt1 of
                          # this block -- pad so its granules start at jt2.
                          wo_gs = ([lambda: None] * 2 +
                                   make_wo_granules(ib - 1, ctxns[ib - 1], ps_aux)
                                   ) if ib > 0 else []
                          norm = attn_block(ib, 1, ctxns[ib], ps_s, ps_ctx, ps_aux,
                                            granules=wo_gs, prev_norm=norm)
                      # tail: last block's norm + its Wo
                      norm(0)
                      norm(1)
                      for g in make_wo_granules(IB - 1, ctxns[IB - 1], ps_aux):
                          g()

    nc.compile()
    return nc


import ml_dtypes as _mld
BF16 = _mld.bfloat16


def _prepare_core_inputs(x, Wq, bq, Wk, bk, Wv, bv, Wo):
    """Shard: core = b*4 + g; batch b, head-group g (channels 256g..256g+256)."""
    in_maps = []
    xTs = [np.ascontiguousarray(np.asarray(x[b]).T.astype(BF16)) for b in range(2)]
    for core in range(8):
        b, g = core // 4, core % 4
        cols = slice(g * DL, (g + 1) * DL)
        in_maps.append({
            "xT": xTs[b],
            "wq": np.ascontiguousarray(Wq[:, cols].astype(BF16)),
            "wk": np.ascontiguousarray(Wk[:, cols].astype(BF16)),
            "wv": np.ascontiguousarray(Wv[:, cols].astype(BF16)),
            "wo": np.ascontiguousarray(Wo[cols, :].astype(BF16)),
            "bq": np.ascontiguousarray(bq[cols].reshape(DT, 128).T),
            "bk": np.ascontiguousarray(bk[cols].reshape(DT, 128).T),
        })
    return in_maps


def kernel(x, Wq, bq, Wk, bk, Wv, bv, Wo, bo, _trace=False, _results_box=None):
    from concourse.bass_utils import run_bass_kernel_spmd

    x = np.asarray(x, dtype=np.float32)
    args = [np.asarray(a, dtype=np.float32) for a in (Wq, bq, Wk, bk, Wv, bv, Wo, bo)]
    Wq, bq, Wk, bk, Wv, bv, Wo, bo = args

    if "nc" not in _prog_cache:
        _prog_cache["nc"] = build_program()
    nc = _prog_cache["nc"]

    in_maps = _prepare_core_inputs(x, Wq, bq, Wk, bk, Wv, bv, Wo)
    res = run_bass_kernel_spmd(nc, in_maps, core_ids=list(range(8)), trace=_trace)
    if _results_box is not None:
        _results_box.append(res)
    parts = [r["y"] for r in res.results]
    out = np.empty((2, N, HID), dtype=np.float32)
    for b in range(2):
        out[b] = parts[4 * b] + parts[4 * b + 1] + parts[4 * b + 2] + parts[4 * b + 3]
    # bv enters linearly: heads(x@Wv + bv) -> ctx rows are convex combos, so
    # softmax(..) @ (v + bv) @ Wo = ctx@Wo + bv@Wo.  Fold bv@Wo + bo here.
    out += bo + bv @ Wo
    return out


# revision 29
# speedup vs baseline: 1.0104x; 1.0104x over previous
"""Trainium2 Bass kernel (bf16 matmul operands, fp32 PSUM accumulation) for multi-head attention (nn_Attention_24764781428921).

Reference (fp32):
    q = heads(x @ Wq + bq); k = heads(x @ Wk + bk); v = heads(x @ Wv + bv)
    probs = softmax(q k^T / sqrt(1024)); ctx = probs @ v
    out = unheads(ctx) @ Wo + bo
with x [2, 2048, 1024], 16 heads, head_dim 64.

Sharding: DP=2 over batch x TP=4 over heads (4 heads / 256 channels per core).
Each core returns a partial [2048, 1024] = ctx_local @ Wo[local_rows]; the host
sums the 4 TP partials per batch and adds bo + bv @ Wo (bv enters linearly:
softmax rows are convex, so probs @ (v + bv) @ Wo = ctx@Wo + bv@Wo).

On-chip layout (per core):
  xT [1024, 2048]  (host pre-transposed)  -> SBUF [128, 8ct, 2048]
  qT/kT = W^T xT + b : [256, 2048] as [128, 2dt, 2048]   (channel on partitions)
  v     = xT^T Wv    : [2048, 256] -> vaug [128, 16jt, 4h, 65] (col 64 = ones)
  s^T(h, jt, ib) [128j, 512i] = matmul(lhsT=kT slice, rhs=qT slice)
     head pairs row-packed (K=64 at partition offsets 0/64 run concurrently)
  exp on ScalarE with scale=1/32 folded in (no max subtraction: |s|<~2)
  ctx^T+sums [65, 512] accumulated over 16 jt (ones row gives softmax denom)
  normalize: recip(sums) -> PE partition-broadcast [64,512]; mul
  out[i, o] partial: matmul(lhsT=ctxn^T [c,128i], rhs=Wo [c, 512o]) accum 2 ct

Scheduling: DMA order is wk/wq/wv/biases then xt per c-tile, wo late, so the
ramp projections pipeline with the arriving tiles.  The ramp runs k(ib0..3) +
q(ib0) chains plus v(0..5) chains in PSUM banks borrowed from the (still idle)
s/ctx pools -- one chain per bank, since start=True zeroes its whole bank.
Remaining projections are emitted densely right after (the tile list scheduler
drops them into PE slack under the exp stream); v(6..15) is fused into the
first attention block.
"""

import numpy as np

HID = 1024
N = 2048
DL = 256          # local channels per core (4 heads x 64)
NHEAD = 4         # local heads
HD = 64
CT = HID // 128   # 8 c-tiles
DT = DL // 128    # 2 d-tiles
JT = N // 128     # 16 j-tiles
IB = N // 512     # 4 i-blocks
SCALE = 1.0 / 32.0  # 1/sqrt(1024)

_prog_cache = {}


def build_program(reps=1, loop_reps=None):
    import concourse.bass as bass
    import concourse.mybir as mybir
    import concourse.tile as tile
    from concourse import bacc

    F32 = mybir.dt.float32
    F32R = mybir.dt.bfloat16  # matmul operand dtype: bf16 gets FWL weight loads
    AF = mybir.ActivationFunctionType

    def mm(out, lhsT, rhs, **kw):
        nc.tensor.matmul(out, lhsT=lhsT, rhs=rhs, **kw)

    nc = bacc.Bacc()
    xT = nc.dram_tensor("xT", [HID, N], F32R, kind="ExternalInput")
    wq = nc.dram_tensor("wq", [HID, DL], F32R, kind="ExternalInput")
    wk = nc.dram_tensor("wk", [HID, DL], F32R, kind="ExternalInput")
    wv = nc.dram_tensor("wv", [HID, DL], F32R, kind="ExternalInput")
    wo = nc.dram_tensor("wo", [DL, HID], F32R, kind="ExternalInput")
    bq = nc.dram_tensor("bq", [128, DT], F32, kind="ExternalInput")
    bk = nc.dram_tensor("bk", [128, DT], F32, kind="ExternalInput")
    y = nc.dram_tensor("y", [N, HID], F32, kind="ExternalOutput")

    with tile.TileContext(nc) as tc:
        with (
            tc.tile_pool(name="consts", bufs=1) as consts,
            tc.tile_pool(name="qkv_sb", bufs=1) as qkv_sb,
            tc.tile_pool(name="exp_sb", bufs=8) as exp_pool,
            tc.tile_pool(name="ctxn_sb", bufs=4) as ctxn_pool,
            tc.tile_pool(name="bcast_sb", bufs=4) as bcast_pool,
            tc.tile_pool(name="rec_sb", bufs=4) as rec_pool,
            tc.tile_pool(name="out_sb", bufs=4) as out_pool,
        ):
            from contextlib import nullcontext
            loop_cm = tc.For_i(0, loop_reps, 1) if loop_reps is not None else nullcontext()
            with loop_cm:
              for rep in range(reps):
                  wk_sb = consts.tile([128, CT, DL], F32R)
                  wq_sb = consts.tile([128, CT, DL], F32R)
                  xt_sb = consts.tile([128, CT, N], F32R)
                  xt_r = xT[:, :].rearrange("(ct p) i -> p ct i", p=128)
                  wv_sb = consts.tile([128, CT, DL], F32R)
                  bq_sb = consts.tile([128, DT], F32)
                  bk_sb = consts.tile([128, DT], F32)
                  wo_sb = consts.tile([128, DT, HID], F32R)
                  nc.sync.dma_start(out=wk_sb, in_=wk[:, :].rearrange("(ct p) d -> p ct d", p=128))
                  nc.sync.dma_start(out=wq_sb, in_=wq[:, :].rearrange("(ct p) d -> p ct d", p=128))
                  nc.sync.dma_start(out=wv_sb, in_=wv[:, :].rearrange("(ct p) d -> p ct d", p=128))
                  nc.sync.dma_start(out=bq_sb, in_=bq[:, :])
                  nc.sync.dma_start(out=bk_sb, in_=bk[:, :])
                  for ct in range(CT):
                      nc.sync.dma_start(out=xt_sb[:, ct, :], in_=xt_r[:, ct, :])

                  ones_sb = consts.tile([1, HD], F32R)
                  nc.vector.memset(ones_sb, 1.0)

                  qT = qkv_sb.tile([128, DT, N], F32R, tag="qT")
                  kT = qkv_sb.tile([128, DT, N], F32R, tag="kT")
                  vaug = qkv_sb.tile([128, JT, NHEAD, HD + 1], F32R, tag="vaug")
                  nc.vector.memset(vaug[:, :, :, HD:HD + 1], 1.0)

                  def proj_ib(w_sb, b_sb, dest, dt, ib, pool, pname):
                      """One (d-tile, i-block) of a q/k projection: an 8-ct PSUM
                      accumulation chain + bias add."""
                      ps = pool.tile([128, 512], F32, tag="pp",
                                     name=f"r{rep}_{pname}_{dt}_{ib}")
                      for ct in range(CT):
                          mm(ps, w_sb[:, ct, dt * 128:(dt + 1) * 128],
                             xt_sb[:, ct, ib * 512:(ib + 1) * 512],
                             start=(ct == 0), stop=(ct == CT - 1))
                      nc.vector.tensor_scalar_add(
                          dest[:, dt, ib * 512:(ib + 1) * 512], ps, b_sb[:, dt:dt + 1])

                  def emit_v(jt, pool):
                      """v[j, d] = xT^T Wv for one j-tile (bias folded on host)."""
                      psv = pool.tile([128, DL], F32, tag="pp", name=f"r{rep}_psv_{jt}")
                      for ct in range(CT):
                          mm(psv, xt_sb[:, ct, jt * 128:(jt + 1) * 128],
                             wv_sb[:, ct, :],
                             start=(ct == 0), stop=(ct == CT - 1))
                      nc.vector.tensor_copy(
                          out=vaug[:, jt, :, 0:HD],
                          in_=psv.rearrange("p (h d) -> p h d", h=NHEAD))

                  def make_norm(ib, hp, ctxn, ctx_ps, ps_bc):
                      """Deferred softmax normalization of a finished block."""
                      def norm(u):
                          poff = u * HD
                          # softmax denominators came out in the ones-row (row HD)
                          rec = rec_pool.tile([1, 512], F32R, tag="rec",
                                              name=f"r{rep}_rec_{ib}_{hp}_{u}")
                          with nc.allow_low_precision(reason="f32r is fp32-width; matmul input rounding"):
                              nc.vector.reciprocal(rec, ctx_ps[u][HD:HD + 1, :])
                          # partition-broadcast via K=1 outer product on the PE
                          bc_ps = ps_bc.tile([HD, 512], F32, tag="bc",
                                             name=f"r{rep}_bcps_{ib}_{hp}_{u}")
                          mm(bc_ps, ones_sb, rec, start=True, stop=True)
                          bc = bcast_pool.tile([HD, 512], F32, tag="bc",
                                               name=f"r{rep}_bc_{ib}_{hp}_{u}")
                          nc.vector.tensor_copy(out=bc, in_=bc_ps)
                          # ctxn = ctx * recip   (bv folded on host)
                          nc.vector.tensor_mul(
                              ctxn[poff:poff + HD, hp, :], ctx_ps[u][0:HD, :], bc)
                      return norm

                  def attn_block(ib, hp, ctxn, ps_s, ps_ctx, ps_bc, v_fn=None):
                      """Scores + exp + ctx accumulate + normalize for one head pair.

                      Emission is software-pipelined one jt ahead so the PE stream
                      never waits in-line on the exp of the current jt."""
                      isl = slice(ib * 512, (ib + 1) * 512)
                      ctx_ps = [
                          ps_ctx.tile([HD + 1, 512], F32, tag="ctx",
                                      name=f"r{rep}_ctx_{ib}_{hp}_{u}")
                          for u in range(2)
                      ]
                      exs = {}
                      for jt in range(JT):
                          if v_fn is not None:
                              v_fn(jt)
                          s_ps = ps_s.tile([128, 2, 512], F32, tag="s",
                                           name=f"r{rep}_s_{ib}_{hp}_{jt}")
                          for u in range(2):
                              poff = u * HD
                              mm(s_ps[:, u, :],
                                 kT[poff:poff + HD, hp, jt * 128:(jt + 1) * 128],
                                 qT[poff:poff + HD, hp, isl],
                                 start=True, stop=True)
                          ex = exp_pool.tile([128, 2, 512], F32R, tag="ex",
                                             name=f"r{rep}_ex_{ib}_{hp}_{jt}")
                          nc.scalar.activation(ex, s_ps, AF.Exp, scale=SCALE)
                          exs[jt] = ex
                          if jt > 0:
                              emit_ctx(ib, hp, ctx_ps, exs.pop(jt - 1), jt - 1)
                      emit_ctx(ib, hp, ctx_ps, exs.pop(JT - 1), JT - 1)
                      norm = make_norm(ib, hp, ctxn, ctx_ps, ps_bc)
                      norm(0)
                      norm(1)

                  def emit_ctx(ib, hp, ctx_ps, ex, jt):
                      for u in range(2):
                          h = 2 * hp + u
                          mm(ctx_ps[u], vaug[:, jt, h, :], ex[:, u, :],
                             start=(jt == 0), stop=(jt == JT - 1))

                  def emit_wo(ib, ctxn, ps_o):
                      for it in range(4):
                          ob = out_pool.tile([128, 2, 512], F32, tag="ob",
                                             name=f"r{rep}_ob_{ib}_{it}")
                          for ot in range(2):
                              pso = ps_o.tile([128, 512], F32, tag="pp",
                                              name=f"r{rep}_pso_{ib}_{it}_{ot}")
                              for ct in range(DT):
                                  mm(pso, ctxn[:, ct, it * 128:(it + 1) * 128],
                                     wo_sb[:, ct, ot * 512:(ot + 1) * 512],
                                     start=(ct == 0), stop=(ct == DT - 1))
                              nc.vector.tensor_copy(out=ob[:, ot, :], in_=pso)
                          nc.gpsimd.dma_start(
                              out=y[ib * 512 + it * 128:ib * 512 + (it + 1) * 128, :],
                              in_=ob.rearrange("p a b -> p (a b)"))

                  def proj_dt(w_sb, b_sb, dest, dt, pool, nblk, pname):
                      """One d-tile of a q/k projection, c-accumulated in PSUM."""
                      for ib0 in range(0, IB, nblk):
                          pss = [
                              pool.tile([128, 512], F32, tag="pss",
                                        name=f"r{rep}_{pname}_{dt}_{ib0 + i}")
                              for i in range(nblk)
                          ]
                          for ct in range(CT):
                              for i in range(nblk):
                                  mm(pss[i],
                                     w_sb[:, ct, dt * 128:(dt + 1) * 128],
                                     xt_sb[:, ct, (ib0 + i) * 512:(ib0 + i + 1) * 512],
                                     start=(ct == 0), stop=(ct == CT - 1))
                          for i in range(nblk):
                              ib = ib0 + i
                              nc.vector.tensor_scalar_add(
                                  dest[:, dt, ib * 512:(ib + 1) * 512],
                                  pss[i], b_sb[:, dt:dt + 1])

                  # ---- phase A: dt0 projections only ----
                  with tc.tile_pool(name="ps_proj", bufs=8, space="PSUM") as ps_proj:
                      proj_dt(wk_sb, bk_sb, kT, 0, ps_proj, IB, "k")
                      proj_dt(wq_sb, bq_sb, qT, 0, ps_proj, IB, "q")

                  # ---- phase B ----
                  with (
                      tc.tile_pool(name="ps_s", bufs=2, space="PSUM") as ps_s,
                      tc.tile_pool(name="ps_ctx", bufs=2, space="PSUM") as ps_ctx,
                      tc.tile_pool(name="ps_bc", bufs=1, space="PSUM") as ps_bc,
                  ):
                      ctxns = {
                          ib: ctxn_pool.tile([128, DT, 512], F32R, tag="ctxn",
                                             name=f"r{rep}_ctxn_{ib}")
                          for ib in range(IB)
                      }

                      # wo lands well before the hp1 blocks; issuing it here
                      # keeps its transfer out of the ramp's xt stream.
                      nc.sync.dma_start(out=wo_sb, in_=wo[:, :].rearrange("(ct p) o -> p ct o", p=128))

                      with tc.tile_pool(name="ps_v", bufs=1, space="PSUM") as ps_v:
                          attn_block(0, 0, ctxns[0], ps_s, ps_ctx, ps_bc,
                                     v_fn=lambda jt: emit_v(jt, ps_v))
                      for ib in range(1, IB):
                          attn_block(ib, 0, ctxns[ib], ps_s, ps_ctx, ps_bc)
                      with tc.tile_pool(name="ps_projB", bufs=1, space="PSUM") as ps_projB:
                          proj_dt(wk_sb, bk_sb, kT, 1, ps_projB, 1, "k")
                          proj_dt(wq_sb, bq_sb, qT, 1, ps_projB, 1, "q")
                      with tc.tile_pool(name="ps_o", bufs=1, space="PSUM") as ps_o:
                          for ib in range(IB):
                              attn_block(ib, 1, ctxns[ib], ps_s, ps_ctx, ps_bc)
                              emit_wo(ib, ctxns[ib], ps_o)

    nc.compile()
    return nc


import ml_dtypes as _mld
BF16 = _mld.bfloat16


def _prepare_core_inputs(x, Wq, bq, Wk, bk, Wv, bv, Wo):
    """Shard: core = b*4 + g; batch b, head-group g (channels 256g..256g+256)."""
    in_maps = []
    xTs = [np.ascontiguousarray(np.asarray(x[b]).T.astype(BF16)) for b in range(2)]
    for core in range(8):
        b, g = core // 4, core % 4
        cols = slice(g * DL, (g + 1) * DL)
        in_maps.append({
            "xT": xTs[b],
            "wq": np.ascontiguousarray(Wq[:, cols].astype(BF16)),
            "wk": np.ascontiguousarray(Wk[:, cols].astype(BF16)),
            "wv": np.ascontiguousarray(Wv[:, cols].astype(BF16)),
            "wo": np.ascontiguousarray(Wo[cols, :].astype(BF16)),
            "bq": np.ascontiguousarray(bq[cols].reshape(DT, 128).T),
            "bk": np.ascontiguousarray(bk[cols].reshape(DT, 128).T),
        })
    return in_maps


def kernel(x, Wq, bq, Wk, bk, Wv, bv, Wo, bo, _trace=False, _results_box=None):
    from concourse.bass_utils import run_bass_kernel_spmd

    x = np.asarray(x, dtype=np.float32)
    args = [np.asarray(a, dtype=np.float32) for a in (Wq, bq, Wk, bk, Wv, bv, Wo, bo)]
    Wq, bq, Wk, bk, Wv, bv, Wo, bo = args

    if "nc" not in _prog_cache:
        _prog_cache["nc"] = build_program()
    nc = _prog_cache["nc"]

    in_maps = _prepare_core_inputs(x, Wq, bq, Wk, bk, Wv, bv, Wo)
    res = run_bass_kernel_spmd(nc, in_maps, core_ids=list(range(8)), trace=_trace)
    if _results_box is not None:
        _results_box.append(res)
    parts = [r["y"] for r in res.results]
    out = np.empty((2, N, HID), dtype=np.float32)
    for b in range(2):
        out[b] = parts[4 * b] + parts[4 * b + 1] + parts[4 * b + 2] + parts[4 * b + 3]
    # bv enters linearly (softmax rows are convex): fold bv @ Wo + bo here.
    out += bo + bv @ Wo
    return out


# revision 33
# speedup vs baseline: 1.0708x; 1.0597x over previous
"""Trainium2 Bass kernel (bf16 matmul operands, fp32 PSUM accumulation) for multi-head attention (nn_Attention_24764781428921).

Reference (fp32):
    q = heads(x @ Wq + bq); k = heads(x @ Wk + bk); v = heads(x @ Wv + bv)
    probs = softmax(q k^T / sqrt(1024)); ctx = probs @ v
    out = unheads(ctx) @ Wo + bo
with x [2, 2048, 1024], 16 heads, head_dim 64.

Sharding: DP=2 over batch x TP=4 over heads (4 heads / 256 channels per core).
Each core returns a partial [2048, 1024] = ctx_local @ Wo[local_rows]; the host
sums the 4 TP partials per batch and adds bo + bv @ Wo (bv enters linearly:
softmax rows are convex, so probs @ (v + bv) @ Wo = ctx@Wo + bv@Wo).

On-chip layout (per core):
  xT [1024, 2048]  (host pre-transposed)  -> SBUF [128, 8ct, 2048]
  qT/kT = W^T xT + b : [256, 2048] as [128, 2dt, 2048]   (channel on partitions)
  v     = xT^T Wv    : [2048, 256] -> vaug [128, 16jt, 4h, 65] (col 64 = ones)
  s^T(h, jt, ib) [128j, 512i] = matmul(lhsT=kT slice, rhs=qT slice)
     head pairs row-packed (K=64 at partition offsets 0/64 run concurrently)
  exp on ScalarE with scale=1/32 folded in (no max subtraction: |s|<~2)
  ctx^T+sums [65, 512] accumulated over 16 jt (ones row gives softmax denom)
  normalize: recip(sums) -> PE partition-broadcast [64,512]; mul
  out[i, o] partial: matmul(lhsT=ctxn^T [c,128i], rhs=Wo [c, 512o]) accum 2 ct

Scheduling: DMA order is wk/wq/wv/biases then xt per c-tile, wo late, so the
ramp projections pipeline with the arriving tiles.  The ramp runs k(ib0..3) +
q(ib0) chains plus v(0..5) chains in PSUM banks borrowed from the (still idle)
s/ctx pools -- one chain per bank, since start=True zeroes its whole bank.
Remaining projections are emitted densely right after (the tile list scheduler
drops them into PE slack under the exp stream); v(6..15) is fused into the
first attention block.
"""

import numpy as np

HID = 1024
N = 2048
DL = 256          # local channels per core (4 heads x 64)
NHEAD = 4         # local heads
HD = 64
CT = HID // 128   # 8 c-tiles
DT = DL // 128    # 2 d-tiles
JT = N // 128     # 16 j-tiles
IB = N // 512     # 4 i-blocks
SCALE = 1.0 / 32.0  # 1/sqrt(1024)

_prog_cache = {}


def build_program(reps=1, loop_reps=None):
    import concourse.bass as bass
    import concourse.mybir as mybir
    import concourse.tile as tile
    from concourse import bacc

    F32 = mybir.dt.float32
    F32R = mybir.dt.bfloat16  # matmul operand dtype: bf16 gets FWL weight loads
    AF = mybir.ActivationFunctionType

    def mm(out, lhsT, rhs, **kw):
        nc.tensor.matmul(out, lhsT=lhsT, rhs=rhs, **kw)

    nc = bacc.Bacc()
    xT = nc.dram_tensor("xT", [HID, N], F32R, kind="ExternalInput")
    wq = nc.dram_tensor("wq", [HID, DL], F32R, kind="ExternalInput")
    wk = nc.dram_tensor("wk", [HID, DL], F32R, kind="ExternalInput")
    wv = nc.dram_tensor("wv", [HID, DL], F32R, kind="ExternalInput")
    wo = nc.dram_tensor("wo", [DL, HID], F32R, kind="ExternalInput")
    bq = nc.dram_tensor("bq", [128, DT], F32, kind="ExternalInput")
    bk = nc.dram_tensor("bk", [128, DT], F32, kind="ExternalInput")
    y = nc.dram_tensor("y", [N, HID], F32, kind="ExternalOutput")

    with tile.TileContext(nc) as tc:
        with (
            tc.tile_pool(name="consts", bufs=1) as consts,
            tc.tile_pool(name="qkv_sb", bufs=1) as qkv_sb,
            tc.tile_pool(name="exp_sb", bufs=8) as exp_pool,
            tc.tile_pool(name="ctxn_sb", bufs=4) as ctxn_pool,
            tc.tile_pool(name="bcast_sb", bufs=4) as bcast_pool,
            tc.tile_pool(name="rec_sb", bufs=4) as rec_pool,
            tc.tile_pool(name="out_sb", bufs=8) as out_pool,
        ):
            from contextlib import nullcontext
            loop_cm = tc.For_i(0, loop_reps, 1) if loop_reps is not None else nullcontext()
            with loop_cm:
              for rep in range(reps):
                  wk_sb = consts.tile([128, CT, DL], F32R)
                  wq_sb = consts.tile([128, CT, DL], F32R)
                  xt_sb = consts.tile([128, CT, N], F32R)
                  xt_r = xT[:, :].rearrange("(ct p) i -> p ct i", p=128)
                  wv_sb = consts.tile([128, CT, DL], F32R)
                  bq_sb = consts.tile([128, DT], F32)
                  bk_sb = consts.tile([128, DT], F32)
                  wo_sb = consts.tile([128, DT, HID], F32R)
                  nc.sync.dma_start(out=wk_sb, in_=wk[:, :].rearrange("(ct p) d -> p ct d", p=128))
                  nc.sync.dma_start(out=wq_sb, in_=wq[:, :].rearrange("(ct p) d -> p ct d", p=128))
                  nc.sync.dma_start(out=wv_sb, in_=wv[:, :].rearrange("(ct p) d -> p ct d", p=128))
                  nc.sync.dma_start(out=bq_sb, in_=bq[:, :])
                  nc.sync.dma_start(out=bk_sb, in_=bk[:, :])
                  for ct in range(CT):
                      nc.sync.dma_start(out=xt_sb[:, ct, :], in_=xt_r[:, ct, :])

                  ones_sb = consts.tile([1, HD], F32R)
                  nc.vector.memset(ones_sb, 1.0)

                  qT = qkv_sb.tile([128, DT, N], F32R, tag="qT")
                  kT = qkv_sb.tile([128, DT, N], F32R, tag="kT")
                  vaug = qkv_sb.tile([128, JT, NHEAD, HD + 1], F32R, tag="vaug")
                  nc.vector.memset(vaug[:, :, :, HD:HD + 1], 1.0)

                  def proj_ib(w_sb, b_sb, dest, dt, ib, pool, pname):
                      """One (d-tile, i-block) of a q/k projection: an 8-ct PSUM
                      accumulation chain + bias add."""
                      ps = pool.tile([128, 512], F32, tag="pp",
                                     name=f"r{rep}_{pname}_{dt}_{ib}")
                      for ct in range(CT):
                          mm(ps, w_sb[:, ct, dt * 128:(dt + 1) * 128],
                             xt_sb[:, ct, ib * 512:(ib + 1) * 512],
                             start=(ct == 0), stop=(ct == CT - 1))
                      nc.vector.tensor_scalar_add(
                          dest[:, dt, ib * 512:(ib + 1) * 512], ps, b_sb[:, dt:dt + 1])

                  def emit_v(jt, pool):
                      """v[j, d] = xT^T Wv for one j-tile (bias folded on host)."""
                      psv = pool.tile([128, DL], F32, tag="pp", name=f"r{rep}_psv_{jt}")
                      for ct in range(CT):
                          mm(psv, xt_sb[:, ct, jt * 128:(jt + 1) * 128],
                             wv_sb[:, ct, :],
                             start=(ct == 0), stop=(ct == CT - 1))
                      nc.vector.tensor_copy(
                          out=vaug[:, jt, :, 0:HD],
                          in_=psv.rearrange("p (h d) -> p h d", h=NHEAD))

                  def make_norm(ib, hp, ctxn, ctx_ps, ps_bc):
                      """Deferred softmax normalization of a finished block."""
                      def norm(u):
                          poff = u * HD
                          # softmax denominators came out in the ones-row (row HD)
                          rec = rec_pool.tile([1, 512], F32R, tag="rec",
                                              name=f"r{rep}_rec_{ib}_{hp}_{u}")
                          with nc.allow_low_precision(reason="f32r is fp32-width; matmul input rounding"):
                              nc.vector.reciprocal(rec, ctx_ps[u][HD:HD + 1, :])
                          # partition-broadcast via K=1 outer product on the PE
                          bc_ps = ps_bc.tile([HD, 512], F32, tag="bc",
                                             name=f"r{rep}_bcps_{ib}_{hp}_{u}")
                          mm(bc_ps, ones_sb, rec, start=True, stop=True)
                          # ctxn = ctx * recip (bv folded on host): evacuate ctx
                          # to SBUF at 2x (bf16 out), then one-PSUM-operand mul
                          # against the broadcast left in PSUM.
                          nc.vector.tensor_copy(
                              out=ctxn[poff:poff + HD, hp, :], in_=ctx_ps[u][0:HD, :])
                          nc.vector.tensor_mul(
                              ctxn[poff:poff + HD, hp, :],
                              ctxn[poff:poff + HD, hp, :], bc_ps)
                      return norm

                  def attn_block(ib, hp, ctxn, ps_s, ps_ctx, ps_bc, v_fn=None):
                      """Scores + exp + ctx accumulate + normalize for one head pair.

                      Emission is software-pipelined one jt ahead so the PE stream
                      never waits in-line on the exp of the current jt."""
                      isl = slice(ib * 512, (ib + 1) * 512)
                      ctx_ps = [
                          ps_ctx.tile([HD + 1, 512], F32, tag="ctx",
                                      name=f"r{rep}_ctx_{ib}_{hp}_{u}")
                          for u in range(2)
                      ]
                      exs = {}
                      for jt in range(JT):
                          if v_fn is not None:
                              v_fn(jt)
                          s_ps = ps_s.tile([128, 2, 512], F32, tag="s",
                                           name=f"r{rep}_s_{ib}_{hp}_{jt}")
                          for u in range(2):
                              poff = u * HD
                              mm(s_ps[:, u, :],
                                 kT[poff:poff + HD, hp, jt * 128:(jt + 1) * 128],
                                 qT[poff:poff + HD, hp, isl],
                                 start=True, stop=True)
                          ex = exp_pool.tile([128, 2, 512], F32R, tag="ex",
                                             name=f"r{rep}_ex_{ib}_{hp}_{jt}")
                          nc.scalar.activation(ex, s_ps, AF.Exp, scale=SCALE)
                          exs[jt] = ex
                          if jt > 0:
                              emit_ctx(ib, hp, ctx_ps, exs.pop(jt - 1), jt - 1)
                      emit_ctx(ib, hp, ctx_ps, exs.pop(JT - 1), JT - 1)
                      norm = make_norm(ib, hp, ctxn, ctx_ps, ps_bc)
                      norm(0)
                      norm(1)

                  def emit_ctx(ib, hp, ctx_ps, ex, jt):
                      for u in range(2):
                          h = 2 * hp + u
                          mm(ctx_ps[u], vaug[:, jt, h, :], ex[:, u, :],
                             start=(jt == 0), stop=(jt == JT - 1))

                  def emit_wo(ib, ctxn, ps_o):
                      for it in range(4):
                          ob = out_pool.tile([128, 2, 512], F32, tag="ob",
                                             name=f"r{rep}_ob_{ib}_{it}")
                          for ot in range(2):
                              pso = ps_o.tile([128, 512], F32, tag="pp",
                                              name=f"r{rep}_pso_{ib}_{it}_{ot}")
                              for ct in range(DT):
                                  mm(pso, ctxn[:, ct, it * 128:(it + 1) * 128],
                                     wo_sb[:, ct, ot * 512:(ot + 1) * 512],
                                     start=(ct == 0), stop=(ct == DT - 1))
                              nc.vector.tensor_copy(out=ob[:, ot, :], in_=pso)
                          nc.gpsimd.dma_start(
                              out=y[ib * 512 + it * 128:ib * 512 + (it + 1) * 128, :],
                              in_=ob.rearrange("p a b -> p (a b)"))

                  def proj_dt(w_sb, b_sb, dest, dt, pool, nblk, pname):
                      """One d-tile of a q/k projection, c-accumulated in PSUM."""
                      for ib0 in range(0, IB, nblk):
                          pss = [
                              pool.tile([128, 512], F32, tag="pss",
                                        name=f"r{rep}_{pname}_{dt}_{ib0 + i}")
                              for i in range(nblk)
                          ]
                          for ct in range(CT):
                              for i in range(nblk):
                                  mm(pss[i],
                                     w_sb[:, ct, dt * 128:(dt + 1) * 128],
                                     xt_sb[:, ct, (ib0 + i) * 512:(ib0 + i + 1) * 512],
                                     start=(ct == 0), stop=(ct == CT - 1))
                          for i in range(nblk):
                              ib = ib0 + i
                              nc.vector.tensor_scalar_add(
                                  dest[:, dt, ib * 512:(ib + 1) * 512],
                                  pss[i], b_sb[:, dt:dt + 1])

                  # ---- phase A: dt0 projections only ----
                  with tc.tile_pool(name="ps_proj", bufs=8, space="PSUM") as ps_proj:
                      proj_dt(wk_sb, bk_sb, kT, 0, ps_proj, IB, "k")
                      proj_dt(wq_sb, bq_sb, qT, 0, ps_proj, IB, "q")

                  # ---- phase B ----
                  with (
                      tc.tile_pool(name="ps_s", bufs=2, space="PSUM") as ps_s,
                      tc.tile_pool(name="ps_ctx", bufs=2, space="PSUM") as ps_ctx,
                      tc.tile_pool(name="ps_bc", bufs=1, space="PSUM") as ps_bc,
                  ):
                      ctxns = {
                          ib: ctxn_pool.tile([128, DT, 512], F32R, tag="ctxn",
                                             name=f"r{rep}_ctxn_{ib}")
                          for ib in range(IB)
                      }

                      # wo lands well before the hp1 blocks; issuing it here
                      # keeps its transfer out of the ramp's xt stream.
                      nc.sync.dma_start(out=wo_sb, in_=wo[:, :].rearrange("(ct p) o -> p ct o", p=128))

                      with tc.tile_pool(name="ps_v", bufs=1, space="PSUM") as ps_v:
                          attn_block(0, 0, ctxns[0], ps_s, ps_ctx, ps_bc,
                                     v_fn=lambda jt: emit_v(jt, ps_v))
                      for ib in range(1, IB):
                          attn_block(ib, 0, ctxns[ib], ps_s, ps_ctx, ps_bc)
                      with tc.tile_pool(name="ps_projB", bufs=1, space="PSUM") as ps_projB:
                          proj_dt(wk_sb, bk_sb, kT, 1, ps_projB, 1, "k")
                          proj_dt(wq_sb, bq_sb, qT, 1, ps_projB, 1, "q")
                      with tc.tile_pool(name="ps_o", bufs=1, space="PSUM") as ps_o:
                          for ib in range(IB):
                              attn_block(ib, 1, ctxns[ib], ps_s, ps_ctx, ps_bc)
                              emit_wo(ib, ctxns[ib], ps_o)

    nc.compile()
    return nc


import ml_dtypes as _mld
BF16 = _mld.bfloat16


def _prepare_core_inputs(x, Wq, bq, Wk, bk, Wv, bv, Wo):
    """Shard: core = b*4 + g; batch b, head-group g (channels 256g..256g+256)."""
    in_maps = []
    xTs = [np.ascontiguousarray(np.asarray(x[b]).T.astype(BF16)) for b in range(2)]
    for core in range(8):
        b, g = core // 4, core % 4
        cols = slice(g * DL, (g + 1) * DL)
        in_maps.append({
            "xT": xTs[b],
            "wq": np.ascontiguousarray(Wq[:, cols].astype(BF16)),
            "wk": np.ascontiguousarray(Wk[:, cols].astype(BF16)),
            "wv": np.ascontiguousarray(Wv[:, cols].astype(BF16)),
            "wo": np.ascontiguousarray(Wo[cols, :].astype(BF16)),
            "bq": np.ascontiguousarray(bq[cols].reshape(DT, 128).T),
            "bk": np.ascontiguousarray(bk[cols].reshape(DT, 128).T),
        })
    return in_maps


def kernel(x, Wq, bq, Wk, bk, Wv, bv, Wo, bo, _trace=False, _results_box=None):
    from concourse.bass_utils import run_bass_kernel_spmd

    x = np.asarray(x, dtype=np.float32)
    args = [np.asarray(a, dtype=np.float32) for a in (Wq, bq, Wk, bk, Wv, bv, Wo, bo)]
    Wq, bq, Wk, bk, Wv, bv, Wo, bo = args

    if "nc" not in _prog_cache:
        _prog_cache["nc"] = build_program()
    nc = _prog_cache["nc"]

    in_maps = _prepare_core_inputs(x, Wq, bq, Wk, bk, Wv, bv, Wo)
    res = run_bass_kernel_spmd(nc, in_maps, core_ids=list(range(8)), trace=_trace)
    if _results_box is not None:
        _results_box.append(res)
    parts = [r["y"] for r in res.results]
    out = np.empty((2, N, HID), dtype=np.float32)
    for b in range(2):
        out[b] = parts[4 * b] + parts[4 * b + 1] + parts[4 * b + 2] + parts[4 * b + 3]
    # bv enters linearly (softmax rows are convex): fold bv @ Wo + bo here.
    out += bo + bv @ Wo
    return out
